# revision 95
# baseline (speedup 1.0000x reference)
"""Trainium2 Bass kernel for nn_AbstractAtt (MLB-style attention + fusion + classifier).

Data-parallel over 8 NeuronCores: batch 128 -> 16 samples/core, weights replicated.

v2 design (vs fp32r v1):
  - main x_v matmul in fp8(e4m3) DoubleRow perf mode: 2 k-subtiles contracted
    per pass (2x PE throughput). Wv host-prescaled by 32 (fp8 subnormal range),
    rescale folded into the activation's scale. v/Wv host-packed into the
    (k-pair, ...) interleaved layout DoubleRow wants, so DMAs are contiguous.
  - v^T for pooling is host-transposed and DMAed (bf16), killing all PE
    v-transposes and their PSUM->SBUF DVE copies.
  - ALL bulk tensors host-packed to their exact SBUF image: one contiguous
    run per partition per DMA (descriptor count == partitions), few DMAs.
  - x_att = tanh(x_v * x_q) via ONE broadcast DVE multiply (bf16, 2x/4x mode)
    + ONE batched ACT tanh per pair, in place, instead of 2 small ACT ops
    per DA tile.
  - everything else bf16 (q, Wq, Wqf, Wa, Wf, Wc streams, xqf, pooling);
    fp8 for any of these fails the 2e-2 gate (tested in numpy).
  - e^T built with 4 PE transposes packed into one PSUM bank (start=False
    accumulate onto the zero region) + one DVE copy.
  - emission order tuned for the in-order engine queues: pair-0's x_v is
    emitted BEFORE the Wq-gated x_q matmuls (PE never head-of-line blocks
    on the Wq stream); v8 software-pipelined 2 pairs ahead; x_v PSUM ring
    3 deep; Wf prefetched during late pairs; Wc streamed at the pl->tl
    pool handoff so the classifier is fed as fusion runs.

Per-core pipeline:
  x_v   = tanh((v8^T @ Wv8) / 32 + bv)    [DA, 2S] per pair, fp8 DoubleRow
  x_att = tanh(x_v * x_q[b])              DVE bcast-mul + ACT tanh (batched)
  scores= Wa^T @ x_att                    [G, 2S] PSUM accum over DA tiles
  att   = exp(scores + ba); row sums via ACT accum_out; softmax denominator
          applied on the pooled output (DVE scalar mul by 1/sum)
  pool  = e^T @ v^T                       v^T host-provided bf16
  xv    = tanh(v_att @ Wf[g] + bf)        glimpse-strided lhsT columns
  x     = tanh(xv * xq);  out = x @ Wc + bc
"""

import os

import ml_dtypes
import numpy as np

NPAIR_OVR = int(os.environ.get("NPAIRS", "0")) or None  # sim-bisect probe

import concourse.bass as bass
import concourse.mybir as mybir
import concourse.tile as tile
from concourse import bacc
from concourse.bass_utils import run_bass_kernel_spmd
from concourse.masks import make_identity

F32 = mybir.dt.float32
BF16 = mybir.dt.bfloat16
F8 = mybir.dt.float8e4
AF = mybir.ActivationFunctionType
DR = mybir.MatmulPerfMode.DoubleRow

# problem constants (hardcoded per contract)
B, DV, W, H = 128, 2048, 14, 14
S = W * H            # 196
DQ = 2048
DA = 1200
G = 4
DH = 2048
DHG = DH // G        # 512
NANS = 3000
NCORES = 8
BPC = B // NCORES    # 16 samples per core
NPAIR = BPC // 2     # 8 pairs

NK = DV // 128       # 16 k-subtiles over DV (== DQ // 128)
KM = 4               # k-subtiles merged per streamed-weight DMA
NKM = NK // KM       # 4 merged groups
KM8 = 4              # k-subtiles per fp8 v/wv SBUF tile (2 DoubleRow pairs)
NKK = NK // KM8      # 4 fp8 tile groups
DA_TILES = [(m * 128, min(128, DA - m * 128)) for m in range((DA + 127) // 128)]
NM = len(DA_TILES)   # 10
S2 = 2 * S           # 392 columns per pair
W8SCALE = 32.0       # host premultiply on Wv before fp8 quant
NANS_TILES = [(j * 500, 500) for j in range(6)]
XQF_TILES = [(j * 256, 256) for j in range(8)]


def build_module(reps: int = 1) -> bacc.Bacc:
    nc = bacc.Bacc("TRN2", target_bir_lowering=False, debug=False)

    # All bulk tensors are host-packed into their exact SBUF image so each
    # DMA is one contiguous run per partition (descriptor count = partitions).
    v8 = nc.dram_tensor("v8", [NPAIR, 128, NKK * KM8 * 2 * S], F8,
                        kind="ExternalInput").ap()
    # host-transposed v for pooling, packed per pair: [128|68, (b, c)] bf16
    vta = nc.dram_tensor("vta", [NPAIR, 128, 2 * DV], BF16,
                         kind="ExternalInput").ap()
    vtb = nc.dram_tensor("vtb", [NPAIR, 68, 2 * DV], BF16,
                         kind="ExternalInput").ap()
    wv8 = nc.dram_tensor("wv8", [128, NKK * KM8 * DA], F8,
                         kind="ExternalInput").ap()
    q = nc.dram_tensor("q", [BPC, DQ], BF16, kind="ExternalInput").ap()
    wq = nc.dram_tensor("wq", [5, 128, NK * 256], BF16,
                        kind="ExternalInput").ap()
    bv = nc.dram_tensor("bv", [DA, 1], F32, kind="ExternalInput").ap()
    bq = nc.dram_tensor("bq", [DA, 1], F32, kind="ExternalInput").ap()
    wa = nc.dram_tensor("wa", [DA, G], BF16, kind="ExternalInput").ap()
    ba = nc.dram_tensor("ba", [G, 1], F32, kind="ExternalInput").ap()
    wf = nc.dram_tensor("wf", [G, 128, NK * DHG], BF16,
                        kind="ExternalInput").ap()
    bfb = nc.dram_tensor("bfb", [1, DH], BF16, kind="ExternalInput").ap()
    wqf = nc.dram_tensor("wqf", [8, 128, NK * 256], BF16,
                         kind="ExternalInput").ap()
    bqf = nc.dram_tensor("bqf", [1, DH], BF16, kind="ExternalInput").ap()
    wc = nc.dram_tensor("wc", [6, 128, NK * 500], BF16,
                        kind="ExternalInput").ap()
    bc = nc.dram_tensor("bc", [1, NANS], BF16, kind="ExternalInput").ap()
    out = nc.dram_tensor("out", [BPC, NANS], F32, kind="ExternalOutput").ap()

    args = (v8, vta, vtb, wv8, q, wq, bv, bq, wa, ba, wf, bfb, wqf, bqf, wc,
            bc, out)
    with tile.TileContext(nc) as tc:
        if reps > 4:
            with tc.For_i(0, reps, 1):
                emit_core(nc, tc, *args)
        else:
            for _ in range(reps):
                emit_core(nc, tc, *args)
    nc.compile()
    return nc


def emit_core(nc, tc, v8, vta, vtb, wv8, q, wq, bv, bq, wa, ba, wf, bfb, wqf,
              bqf, wc, bc, out):
    from contextlib import ExitStack

    ctx = ExitStack()
    with ctx:
        # ---------------- persistent pools ----------------
        const_pool = ctx.enter_context(tc.tile_pool(name="const", bufs=1))

        ident = const_pool.tile([128, 128], BF16)
        make_identity(nc, ident[:])
        ones = const_pool.tile([1, BPC], BF16)
        nc.gpsimd.memset(ones[:], 1.0)

        # per-partition bias tiles, packed loads (col m = DA tile m)
        bv_sb = const_pool.tile([128, NM], F32)
        bq_sb = const_pool.tile([128, NM], F32)
        nc.sync.dma_start(bv_sb[:, :9], bv[0:1152, 0].rearrange("(m p) -> p m", p=128))
        nc.sync.dma_start(bv_sb[:48, 9:10], bv[1152:1200, :])
        nc.sync.dma_start(bq_sb[:, :9], bq[0:1152, 0].rearrange("(m p) -> p m", p=128))
        nc.sync.dma_start(bq_sb[:48, 9:10], bq[1152:1200, :])
        ba_sb = const_pool.tile([G, 1], F32)
        nc.sync.dma_start(ba_sb[:], ba[:])
        # wa packed: [128, 40] bf16, cols m*4..m*4+4 = Wa rows m*128..+128
        wa_sb = const_pool.tile([128, G * NM], BF16)
        nc.sync.dma_start(
            wa_sb[:, :36].rearrange("p (m g) -> p m g", g=G),
            wa[0:1152, :].rearrange("(m p) g -> p m g", p=128))
        nc.sync.dma_start(wa_sb[:48, 36:40], wa[1152:1200, :])
        bqf_sb = const_pool.tile([1, DH], BF16)
        nc.sync.dma_start(bqf_sb[:], bqf[:])

        # xqf accumulator [BPC, DH] bf16, chunks interleaved with pair loop
        xqf_sb = const_pool.tile([BPC, DH], BF16)
        # v_att collection [BPC*G, DV] bf16 (partition = 4*b + g)
        vatt_sb = const_pool.tile([G * BPC, DV], BF16)

        # hoisted weight-stream pool so Wf prefetch can start in the pair loop
        wfp = ctx.enter_context(tc.tile_pool(name="wfp", bufs=1))
        wf_tiles = {}

        def load_wf(g):
            t = wfp.tile([128, NK * DHG], BF16, tag=f"wf{g}")
            nc.sync.dma_start(t[:], wf[g])
            wf_tiles[g] = t

        wc_tiles = {}

        # ---------------- pre-phase + pair loop (shared SBUF pool) ---------
        with tc.tile_pool(name="pl", bufs=1) as pl:

            # resident fp8 Wv tile (freed with the pair-loop pool); its DMA
            # is emitted inside the pre block AFTER the tiny q load so qT
            # transposes aren't queued behind the 2.4 MB transfer
            wv8_sb = pl.tile([128, NKK * KM8 * DA], F8, tag="wv8")
            wv8_r = wv8_sb[:].rearrange("p (kk k d) -> p kk k d",
                                        kk=NKK, k=KM8)

            npairs = NPAIR_OVR or NPAIR
            v8_tiles = {}

            def load_v8(p):
                # fp8 v tile [128, (kk, ki, b, s)], ONE contiguous DMA,
                # software-pipelined 2 pairs ahead
                t = pl.tile([128, NKK, KM8, 2, S], F8, tag="v8", bufs=3)
                nc.sync.dma_start(
                    t[:].rearrange("p a b c s -> p (a b c s)"), v8[p])
                v8_tiles[p] = t

            def load_vt(pair):
                # v^T tiles for pooling (bf16, host-packed); both samples
                # of the pair share one tile (dim1 = sample)
                vt0 = pl.tile([128, 2, DV], BF16, tag="vt0", bufs=2)
                nc.sync.dma_start(vt0[:].rearrange("p b c -> p (b c)"),
                                  vta[pair])
                vt1 = pl.tile([68, 2, DV], BF16, tag="vt1", bufs=2)
                nc.sync.dma_start(vt1[:].rearrange("p b c -> p (b c)"),
                                  vtb[pair])
                return vt0, vt1

            def emit_xv(pair, ps_pool, pm_tag, prefetch=True):
                if prefetch:
                    if pair + 2 < npairs:
                        load_v8(pair + 2)
                    vt0, vt1 = load_vt(pair)
                else:
                    vt0 = vt1 = None
                v8t = v8_tiles.pop(pair)

                # x_v = tanh(mm/W8SCALE + bv), DoubleRow fp8 accumulation
                xv_all = pl.tile([128, NM * S2], BF16, tag="xv", bufs=2)
                for m, (m0, mw) in enumerate(DA_TILES):
                    pm = ps_pool.tile([128, S2], F32, tag=pm_tag, bufs=3)
                    for kk in range(NKK):
                        for t2 in range(KM8 // 2):
                            nc.tensor.matmul(
                                pm[:mw, :],
                                wv8_r[:, kk, 2 * t2:2 * t2 + 2, m0:m0 + mw],
                                v8t[:, kk, 2 * t2:2 * t2 + 2, :, :],
                                start=(kk == 0 and t2 == 0),
                                stop=(kk == NKK - 1 and t2 == KM8 // 2 - 1),
                                perf_mode=DR, skip_group_check=True)
                    nc.scalar.activation(xv_all[:mw, m * S2:(m + 1) * S2],
                                         pm[:mw, :], AF.Tanh,
                                         bias=bv_sb[:mw, m:m + 1],
                                         scale=1.0 / W8SCALE)
                return xv_all, vt0, vt1

            def emit_rest(pair, xv_all, vt0, vt1):
                b0 = pair * 2
                # x_att = tanh(x_v * x_q[b]): bcast DVE mul + ACT tanh,
                # both in-place over xv_all (elementwise, saves SBUF)
                in0 = xv_all[:].rearrange("p (m b s) -> p m b s", m=NM, b=2)
                in1 = (xqT[:].rearrange("p (m b) -> p m b", m=NM)
                       [:, :, b0:b0 + 2].unsqueeze(3)
                       .broadcast_to([128, NM, 2, S]))
                nc.vector.tensor_mul(in0, in0, in1)
                nc.scalar.activation(xv_all[:], xv_all[:], AF.Tanh)
                xa_all = xv_all

                # scores accumulation over DA tiles
                p_sc = pl_ps.tile([G, S2], F32, tag="psc", bufs=1)
                for m, (m0, mw) in enumerate(DA_TILES):
                    nc.tensor.matmul(p_sc[:], wa_sb[:mw, m * G:(m + 1) * G],
                                     xa_all[:mw, m * S2:(m + 1) * S2],
                                     start=(m == 0), stop=(m == NM - 1),
                                     skip_group_check=True)

                # att = exp(scores + ba) with per-sample row sums
                e_sb = pl.tile([G, S2], BF16, tag="e", bufs=2)
                esum = pl.tile([G, 2], F32, tag="esum", bufs=2)
                for s in range(2):
                    nc.scalar.activation(e_sb[:, s * S:(s + 1) * S],
                                         p_sc[:, s * S:(s + 1) * S], AF.Exp,
                                         bias=ba_sb[:],
                                         accum_out=esum[:, s:s + 1])
                recip = pl.tile([G, 2], F32, tag="recip", bufs=2)
                nc.vector.reciprocal(recip[:], esum[:])

                # e^T: 4 transposes packed into one PSUM bank + one DVE copy
                # cols (s, half, g); first transpose zeroes the whole bank.
                peT = pl_ps.tile([128, 4 * G], BF16, tag="peT", bufs=1)
                first = True
                for s in range(2):
                    nc.tensor.matmul(peT[:, s * 2 * G:s * 2 * G + G],
                                     e_sb[:, s * S:s * S + 128],
                                     ident[:G, :G], is_transpose=True,
                                     start=first, stop=False,
                                     skip_group_check=True)
                    first = False
                    nc.tensor.matmul(peT[:68, s * 2 * G + G:s * 2 * G + 2 * G],
                                     e_sb[:, s * S + 128:(s + 1) * S],
                                     ident[:G, :G], is_transpose=True,
                                     start=False, stop=(s == 1),
                                     skip_group_check=True)
                eT = pl.tile([128, 4 * G], BF16, tag="eT", bufs=2)
                nc.vector.tensor_copy(eT[:], peT[:])

                # pooling: U[g, c] = e^T @ v^T; normalize into tmp; 1 DMA to
                # vatt (DVE writes need 32-aligned partition base, DMA not)
                for s in range(2):
                    tmp = pl.tile([G, DV], BF16, tag="ptmp", bufs=2)
                    for c in range(DV // 512):
                        c0 = c * 512
                        pp = pl_ps.tile([G, 512], F32, tag="ppool", bufs=2)
                        nc.tensor.matmul(pp[:],
                                         eT[:, s * 2 * G:s * 2 * G + G],
                                         vt0[:, s, c0:c0 + 512],
                                         start=True, stop=False,
                                         skip_group_check=True)
                        nc.tensor.matmul(
                            pp[:], eT[:68, s * 2 * G + G:s * 2 * G + 2 * G],
                            vt1[:, s, c0:c0 + 512],
                            start=False, stop=True, skip_group_check=True)
                        nc.vector.tensor_scalar_mul(
                            tmp[:, c0:c0 + 512], pp[:], recip[:, s:s + 1])
                    nc.sync.dma_start(
                        vatt_sb[(b0 + s) * G:(b0 + s + 1) * G, :], tmp[:])

            # ---- pre-phase A: q^T (needs only the tiny q DMA), then pair-0
            # x_v (needs wv8+v8[0]), then pre-phase B (x_q, gated on the Wq
            # stream) — so the PE queue never head-of-line blocks on Wq.
            with tc.tile_pool(name="pre", bufs=1) as pre, \
                 tc.tile_pool(name="pre_ps", bufs=1, space="PSUM") as pre_ps:
                q_sb = pre.tile([BPC, DQ], BF16)
                nc.sync.dma_start(q_sb[:], q[:])
                nc.sync.dma_start(wv8_sb[:], wv8[:])
                load_v8(0)
                if npairs > 1:
                    load_v8(1)
                # qT: [DQ(k-subtiles), BPC] bf16, one tile [128, NK*16]
                qT = const_pool.tile([128, NK * BPC], BF16)
                for k in range(NK):
                    p = pre_ps.tile([128, BPC], BF16, tag="qt", bufs=2)
                    nc.tensor.transpose(p[:], q_sb[:, k * 128:(k + 1) * 128],
                                        ident[:BPC, :BPC])
                    nc.vector.tensor_copy(qT[:, k * BPC:(k + 1) * BPC], p[:])

                xv0, _, _ = emit_xv(0, pre_ps, "pmain0", prefetch=False)

                # x_q_lin = q @ Wq; Wq host-padded to 5 chunks of 256 cols so
                # the stream shares the hoisted "wqf" tag (and space)
                xq_lin = pre.tile([BPC, DA], BF16)
                for j in range(5):
                    n0 = j * 256
                    valid = min(256, DA - n0)
                    pj = pre_ps.tile([BPC, 256], F32, tag="xq", bufs=2)
                    wt = pl.tile([128, NK * 256], BF16, tag="wqfs", bufs=2)
                    nc.scalar.dma_start(wt[:], wq[j])
                    for k in range(NK):
                        nc.tensor.matmul(pj[:], qT[:, k * BPC:(k + 1) * BPC],
                                         wt[:, k * 256:(k + 1) * 256],
                                         start=(k == 0), stop=(k == NK - 1))
                    nc.vector.tensor_copy(xq_lin[:, n0:n0 + valid],
                                          pj[:, :valid])

                # x_qT[m] = tanh(xq_lin^T + bq) per DA tile -> [mw, BPC] bf16
                xqT = const_pool.tile([128, NM * BPC], BF16)
                for m, (m0, mw) in enumerate(DA_TILES):
                    p = pre_ps.tile([128, BPC], BF16, tag="qt", bufs=2)
                    nc.tensor.transpose(p[:mw, :], xq_lin[:, m0:m0 + mw],
                                        ident[:BPC, :BPC])
                    nc.scalar.activation(xqT[:mw, m * BPC:(m + 1) * BPC],
                                         p[:mw, :], AF.Tanh,
                                         bias=bq_sb[:mw, m:m + 1])

            pair_ctx = ExitStack()
            pl_ps = pair_ctx.enter_context(
                tc.tile_pool(name="pl_ps", bufs=1, space="PSUM"))

            def xqf_chunk(j):
                n0, nw = XQF_TILES[j]
                pj = pl_ps.tile([BPC, nw], F32, tag="pqf", bufs=1)
                wt = pl.tile([128, NK * nw], BF16, tag="wqfs", bufs=2)
                nc.sync.dma_start(wt[:], wqf[j])
                for k in range(NK):
                    nc.tensor.matmul(pj[:], qT[:, k * BPC:(k + 1) * BPC],
                                     wt[:, k * nw:(k + 1) * nw],
                                     start=(k == 0), stop=False,
                                     skip_group_check=True)
                nc.tensor.matmul(pj[:], ones[:], bqf_sb[:, n0:n0 + nw],
                                 start=False, stop=True, skip_group_check=True)
                nc.scalar.activation(xqf_sb[:, n0:n0 + nw], pj[:], AF.Tanh)

            # pair-0's deferred loads go behind the Wq stream on purpose
            if npairs > 2:
                load_v8(2)
            vt0_0, vt1_0 = load_vt(0)
            emit_rest(0, xv0, vt0_0, vt1_0)
            xqf_chunk(0)
            for pair in range(1, npairs):
                xv_p, vt0_p, vt1_p = emit_xv(pair, pl_ps, "pmain")
                emit_rest(pair, xv_p, vt0_p, vt1_p)
                xqf_chunk(pair)
                # prefetch Wf during late pairs (consumed at tail start)
                if pair >= 4:
                    load_wf(pair - 4)
            for j in range(npairs, 8):   # no-op unless NPAIRS probe active
                xqf_chunk(j)
            for g2 in range(G):
                if g2 not in wf_tiles:
                    load_wf(g2)
            for g2 in range(G):          # no-op unless NPAIRS probe active
                if g2 not in wf_tiles:
                    load_wf(g2)
            pair_ctx.close()             # free pair-loop PSUM before tail

        # ---------------- tail: vaT transpose, fusion, xqf, classifier -----
        with tc.tile_pool(name="tl", bufs=1) as tl, \
             tc.tile_pool(name="tl_ps", bufs=1, space="PSUM") as tl_ps:
            bf_sb = tl.tile([1, DH], BF16)
            nc.sync.dma_start(bf_sb[:], bfb[:])
            bc_sb = tl.tile([1, NANS], BF16)
            nc.sync.dma_start(bc_sb[:], bc[:])

            # stream all Wc immediately (classifier consumes later)
            for j in range(len(NANS_TILES)):
                t = tl.tile([128, NK * 500], BF16, tag=f"wc{j}", bufs=1)
                nc.sync.dma_start(t[:], wc[j])
                wc_tiles[j] = t

            # vaT[k]: [128, (b, g)] bf16; fusion uses strided per-g columns
            vaT = []
            for k in range(NK):
                p = tl_ps.tile([128, G * BPC], BF16, tag="pvat", bufs=2)
                nc.tensor.transpose(p[:], vatt_sb[:, k * 128:(k + 1) * 128],
                                    ident[:G * BPC, :G * BPC])
                t = tl.tile([128, G * BPC], BF16, tag=f"vaT{k}")
                nc.vector.tensor_copy(t[:], p[:])
                vaT.append(t)

            xv_sb = tl.tile([BPC, DH], BF16)
            out_sb = tl.tile([BPC, NANS], F32)
            for g in range(G):
                pd = tl_ps.tile([BPC, DHG], F32, tag="pd", bufs=2)
                for k in range(NK):
                    lhs = (vaT[k][:].rearrange("p (b g) -> p b g", g=G)
                           [:, :, g])
                    nc.tensor.matmul(
                        pd[:], lhs,
                        wf_tiles[g][:, k * DHG:(k + 1) * DHG],
                        start=(k == 0), stop=False, skip_group_check=True)
                nc.tensor.matmul(pd[:], ones[:],
                                 bf_sb[:, g * DHG:(g + 1) * DHG],
                                 start=False, stop=True, skip_group_check=True)
                nc.scalar.activation(xv_sb[:, g * DHG:(g + 1) * DHG], pd[:],
                                     AF.Tanh)
            # x = tanh(xv * xqf) transposed into xT[k] tiles (bf16)
            xT = []
            for k in range(NK):
                xmk = tl.tile([BPC, 128], BF16, tag="xmk", bufs=3)
                nc.vector.tensor_mul(xmk[:], xv_sb[:, k * 128:(k + 1) * 128],
                                     xqf_sb[:, k * 128:(k + 1) * 128])
                px = tl_ps.tile([128, BPC], BF16, tag="pxT", bufs=2)
                nc.tensor.transpose(px[:], xmk[:], ident[:BPC, :BPC])
                xTk = tl.tile([128, BPC], BF16, tag=f"xT{k}")
                nc.scalar.activation(xTk[:], px[:], AF.Tanh)
                xT.append(xTk)
            # classifier j-outer over streamed Wc tiles
            for j, (n0, nw) in enumerate(NANS_TILES):
                pc = tl_ps.tile([BPC, nw], F32, tag="pc", bufs=2)
                wct = wc_tiles[j]
                for k in range(NK):
                    nc.tensor.matmul(pc[:], xT[k][:],
                                     wct[:, k * nw:(k + 1) * nw],
                                     start=(k == 0), stop=False,
                                     skip_group_check=True)
                nc.tensor.matmul(pc[:], ones[:], bc_sb[:, n0:n0 + nw],
                                 start=False, stop=True, skip_group_check=True)
                nc.vector.tensor_copy(out_sb[:, n0:n0 + nw], pc[:])
            nc.sync.dma_start(out[:], out_sb[:])


_module_cache = {}


def _get_module(reps: int = 1):
    if reps not in _module_cache:
        _module_cache[reps] = build_module(reps)
    return _module_cache[reps]


def make_in_maps(inputs: dict) -> list:
    F8NP = ml_dtypes.float8_e4m3
    BFNP = ml_dtypes.bfloat16
    iv = np.ascontiguousarray(inputs["input_v"], np.float32).reshape(B, DV, S)
    xq = np.ascontiguousarray(inputs["x_q_vec"], np.float32)

    # Wv: scale, quantize fp8, pack [128, (kk, ki, d)]
    wv_s = (np.asarray(inputs["Wv_att"], np.float32) * W8SCALE).astype(F8NP)
    wv_pk = np.ascontiguousarray(
        wv_s.reshape(NKK, KM8, 128, DA).transpose(2, 0, 1, 3)
    ).reshape(128, NKK * KM8 * DA)

    def pack_stream(w, nchunks, nw):
        # [K, N] bf16 -> [nchunks, 128, NK*nw]: chunk j partition p holds
        # (k, n) contiguous for columns j*nw..(j+1)*nw
        w = np.asarray(w).astype(BFNP)
        kp = w.reshape(NK, 128, w.shape[1])          # [k, p, n]
        return np.ascontiguousarray(
            kp.transpose(1, 0, 2).reshape(128, NK, nchunks, nw)
            .transpose(2, 0, 1, 3)).reshape(nchunks, 128, NK * nw)

    wf_np = np.asarray(inputs["Wf"]).astype(BFNP)    # [G, DV, DHG]
    wf_pk = np.ascontiguousarray(
        wf_np.reshape(G, NK, 128, DHG).transpose(0, 2, 1, 3)
    ).reshape(G, 128, NK * DHG)

    shared = {
        "wv8": wv_pk,
        "bv": np.ascontiguousarray(inputs["bv_att"], np.float32).reshape(DA, 1),
        "wq": pack_stream(
            np.pad(np.asarray(inputs["Wq_att"], np.float32),
                   ((0, 0), (0, 5 * 256 - DA))), 5, 256),
        "bq": np.ascontiguousarray(inputs["bq_att"], np.float32).reshape(DA, 1),
        "wa": np.asarray(inputs["Wa"]).astype(BFNP),
        "ba": np.ascontiguousarray(inputs["ba"], np.float32).reshape(G, 1),
        "wf": wf_pk,
        "bfb": np.asarray(inputs["bf"]).astype(BFNP).reshape(1, DH),
        "wqf": pack_stream(inputs["Wqf"], 8, 256),
        "bqf": np.asarray(inputs["bqf"]).astype(BFNP).reshape(1, DH),
        "wc": pack_stream(inputs["Wc"], 6, 500),
        "bc": np.asarray(inputs["bc"]).astype(BFNP).reshape(1, NANS),
    }
    in_maps = []
    for c in range(NCORES):
        vv = iv[c * BPC:(c + 1) * BPC]                       # [BPC, DV, S]
        v8c = vv.astype(F8NP)
        # [NPAIR, 128, (kk, ki, b, s)]
        v8p = np.ascontiguousarray(
            v8c.reshape(NPAIR, 2, NKK, KM8, 128, S).transpose(0, 4, 2, 3, 1, 5)
        ).reshape(NPAIR, 128, NKK * KM8 * 2 * S)
        vtc = vv.transpose(0, 2, 1).astype(BFNP)             # [BPC, S, DV]
        vtc = vtc.reshape(NPAIR, 2, S, DV)
        vta_pk = np.ascontiguousarray(
            vtc[:, :, 0:128, :].transpose(0, 2, 1, 3)
        ).reshape(NPAIR, 128, 2 * DV)
        vtb_pk = np.ascontiguousarray(
            vtc[:, :, 128:S, :].transpose(0, 2, 1, 3)
        ).reshape(NPAIR, 68, 2 * DV)
        m = dict(shared)
        m["v8"] = v8p
        m["vta"] = vta_pk
        m["vtb"] = vtb_pk
        m["q"] = xq[c * BPC:(c + 1) * BPC].astype(BFNP)
        in_maps.append(m)
    return in_maps


def kernel(**inputs) -> np.ndarray:
    nc = _get_module(1)
    in_maps = make_in_maps(inputs)
    res = run_bass_kernel_spmd(nc, in_maps, core_ids=list(range(NCORES)))
    return np.concatenate([res.results[c]["out"] for c in range(NCORES)], axis=0)
